# revision 9
# baseline (speedup 1.0000x reference)
"""Trainium2 Bass kernel for nn_AttnGlobal (B=8, N=4096, DIM=128).

reference:
    kv = x @ Wkv + bkv ; k, v = split(kv)
    q = q_global / sqrt(d)
    scores = einsum("bnd,bmd->bnm", k, q)       # softmax over m
    attn = softmax(scores, axis=-1)
    out = einsum("bnm,bmd->bnd", attn, v) @ Wp + bp

Sharding: pure data-parallel over B across the 8 cores (one batch each).

Host-side algebra folds:
    w   = x @ (Wv @ Wp)            (since attn @ (x@Wv) @ Wp = attn @ (x@(Wv@Wp)))
    bpe = bv @ Wp + bp             (since rows of attn sum to 1)

Per-core dataflow:
    xT, qT  : host-pretransposed fp16 inputs        [d, n] / [d, m]
    kT      = Wk.T @ xT + bk                        [d, n]   fp16
    S.T     = qT.T-chunks @ kT                      [m, n] tiles in PSUM (fp32)
    E.T     = exp(S.T / sqrt(d))                    fp16, ACT straight from PSUM
    U_aug   = E @ [w | 1]                           [n, 129] accumulated in PSUM
    out     = U[:, :128] * (1 / U[:, 128]) + bpe    DVE, then DMA out

Schedule: the ACT engine's exp stream (16.8M elems/core @ 1 elem/cyc/lane)
is the hard floor (~133us). The software pipeline keeps ACT gapless:
per chunk c we emit S-groups of chunk c+1 interleaved with U-batches of
chunk c (which consume exp output written one chunk-iteration earlier),
so the PE fills its ACT-slot-wait gaps with U matmuls instead of idling.
DMA triggers live on sync/gpsimd queues (never the scalar engine).
"""

import os
import sys

try:
    import concourse  # noqa: F401  (resolvable via PYTHONPATH on axon images)
except ImportError:
    for _p in ("/opt/trn_rl_repo", os.path.expanduser("~/.axon_site/_ro/trn_rl_repo")):
        if os.path.isdir(_p) and _p not in sys.path:
            sys.path.append(_p)

import numpy as np

import concourse.bacc as bacc
import concourse.mybir as mybir
from concourse.bass_utils import run_bass_kernel_spmd
from concourse.tile import TileContext

B, N, D = 8, 4096, 128
NT = N // 128          # 32 row tiles
NC = N // 512          # 8 column chunks
F32 = mybir.dt.float32
F16 = mybir.dt.float16
EXP_SCALE = 1.0 / float(np.sqrt(D))

# alternating PSUM score-group sizes; sum == NT, st4 uses 4 banks, st2 uses 2
S_GROUPS = [2, 4, 2, 4, 2, 4, 2, 4, 2, 4, 2]
assert sum(S_GROUPS) == NT
S_STARTS = [sum(S_GROUPS[:i]) for i in range(len(S_GROUPS))]


def build(reps: int = 1):
    """Build and compile the per-core Bass program (identical on all cores)."""
    nc = bacc.Bacc("TRN2", target_bir_lowering=False)

    xt = nc.dram_tensor("xt", [D, N], F16, kind="ExternalInput")
    qt = nc.dram_tensor("qt", [D, N], F16, kind="ExternalInput")
    wk = nc.dram_tensor("wk", [D, D], F16, kind="ExternalInput")
    wvp = nc.dram_tensor("wvp", [D, D], F16, kind="ExternalInput")
    bk = nc.dram_tensor("bk", [D, 1], F32, kind="ExternalInput")
    bpe = nc.dram_tensor("bpe", [D, D], F32, kind="ExternalInput")  # row-tiled bias
    out = nc.dram_tensor("out", [N, D], F32, kind="ExternalOutput")

    with TileContext(nc) as tc:
        xTc = [nc.alloc_sbuf_tensor(f"xT{c}", [128, 512], F16) for c in range(NC)]
        qTc = [nc.alloc_sbuf_tensor(f"qT{c}", [128, 512], F16) for c in range(NC)]
        kTc = [nc.alloc_sbuf_tensor(f"kT{c}", [128, 512], F16) for c in range(NC)]
        w_aug = nc.alloc_sbuf_tensor("w_aug", [128, NT, 130], F16)
        ET = [nc.alloc_sbuf_tensor(f"et{i}", [128, NT, 512], F16) for i in range(2)]
        warm_sb = nc.alloc_sbuf_tensor("warm_sb", [128, 128], F16)
        wk_sb = nc.alloc_sbuf_tensor("wk_sb", [128, 128], F16)
        wvp_sb = nc.alloc_sbuf_tensor("wvp_sb", [128, 128], F16)
        bk_sb = nc.alloc_sbuf_tensor("bk_sb", [128, 1], F32)
        bpe_sb = nc.alloc_sbuf_tensor("bpe_sb", [128, 128], F32)

        # weights ride the (idle-at-startup) scalar queue so xt/qt triggers
        # go out first on sync/gpsimd
        nc.scalar.dma_start(wk_sb[:], wk[:])
        nc.scalar.dma_start(bk_sb[:], bk[:])
        nc.scalar.dma_start(wvp_sb[:], wvp[:])
        nc.scalar.dma_start(bpe_sb[:], bpe[:])

        with (
            tc.tile_pool(name="outp", bufs=4) as outp,
            tc.tile_pool(name="small", bufs=4) as small,
            tc.tile_pool(name="ps", bufs=2, space="PSUM") as psh,
            tc.tile_pool(name="st4", bufs=1, space="PSUM") as st4,
            tc.tile_pool(name="st2", bufs=1, space="PSUM") as st2,
        ):
            uacc = {}

            def s_group(c, mt, g, pool=None, tag=None):
                """scores S.T [m-tiles mt..mt+g, n-chunk c] -> exp -> E.T"""
                if pool is None:
                    pool, tag = (st4, "st4") if g == 4 else (st2, "st2")
                stp = pool.tile([128, g * 512], F32, tag=tag)
                for i in range(g):
                    m = mt + i
                    nc.tensor.matmul(
                        stp[:, i * 512:(i + 1) * 512],
                        qTc[m // 4][:, (m % 4) * 128:(m % 4 + 1) * 128],
                        kTc[c][:],
                    )
                nc.scalar.activation(
                    ET[c % 2][:, mt:mt + g, :],
                    stp[:],
                    mybir.ActivationFunctionType.Exp,
                    scale=EXP_SCALE,
                )

            def u_batch(c, mt, g):
                """U += E.T-tiles[mt..mt+g].T @ [w|1] for output chunk c."""
                if c not in uacc:
                    upa = psh.tile([128, 512], F32, tag="ps")
                    upb = psh.tile([128, 512], F32, tag="ps")
                    uacc[c] = (upa, upb)
                ups = uacc[c]
                buf = ET[c % 2]
                for i in range(g):
                    t = mt + i
                    for j in range(4):
                        up = ups[j // 2]
                        off = 129 * (j % 2)
                        nc.tensor.matmul(
                            up[:, off:off + 129],
                            buf[:, t, j * 128:(j + 1) * 128],
                            w_aug[:, t, :129],
                            start=(t == 0 and j % 2 == 0),
                            stop=(t == NT - 1 and j % 2 == 1),
                        )

            def u_final(c):
                """normalize U by its ones-column, add bias, DMA out."""
                ups = uacc.pop(c)
                for j in range(4):
                    up = ups[j // 2]
                    off = 129 * (j % 2)
                    rec = small.tile([128, 1], F32, tag="rec")
                    nc.vector.reciprocal(rec[:], up[:, off + 128:off + 129])
                    ot = outp.tile([128, 128], F32, tag="ot")
                    nc.vector.scalar_tensor_tensor(
                        ot[:],
                        up[:, off:off + 128],
                        rec[:],
                        bpe_sb[:],
                        mybir.AluOpType.mult,
                        mybir.AluOpType.add,
                    )
                    row = c * 512 + j * 128
                    nc.sync.dma_start(out[row:row + 128, :], ot[:])

            def body(_iv=None):
                # HAM warmup: data-independent matmuls into a scratch PSUM
                # slot keep the PE busy while the first input DMAs land, so
                # the 4096-cycle activity window un-throttles the clock gate
                # (K=4/8 -> 8/8) before the real S-matmuls start.
                nc.vector.memset(warm_sb[:], 0.0)
                warm = psh.tile([128, 512], F32, tag="ps")
                for _ in range(6):
                    nc.tensor.matmul(warm[:, :128], warm_sb[:], warm_sb[:])

                # phase 1: stream xT/qT chunks (sync + gpsimd HWDGE queues);
                # kT + w_aug per chunk; S(0) groups ride as qT tiles arrive.
                nc.vector.memset(w_aug[:, :, 128:129], 1.0)
                sg = 0
                mt_done = 0
                for c in range(NC):
                    nc.sync.dma_start(xTc[c][:], xt[:, c * 512:(c + 1) * 512])
                    nc.gpsimd.dma_start(qTc[c][:], qt[:, c * 512:(c + 1) * 512])
                    kt = psh.tile([128, 512], F32, tag="ps")
                    nc.tensor.matmul(kt[:], wk_sb[:], xTc[c][:])
                    nc.vector.tensor_scalar_add(kTc[c][:], kt[:], bk_sb[:])
                    while sg < len(S_GROUPS) and mt_done + S_GROUPS[sg] <= (c + 1) * 4:
                        s_group(0, mt_done, S_GROUPS[sg])
                        mt_done += S_GROUPS[sg]
                        sg += 1
                    for i in range(4):
                        t = c * 4 + i
                        wp = psh.tile([128, 512], F32, tag="ps")
                        nc.tensor.matmul(
                            wp[:, :128],
                            xTc[c][:, i * 128:(i + 1) * 128],
                            wvp_sb[:],
                        )
                        nc.vector.tensor_copy(w_aug[:, t, :128], wp[:, :128])

                # main pipeline: iteration c emits S-groups of chunk c+1
                # interleaved with U-batches of chunk c (whose exp output
                # was produced during iteration c-1 / phase 1), keeping the
                # ACT exp stream gapless while the PE alternates S and U.
                for c in range(NC - 2):
                    for i in range(len(S_GROUPS) + 1):
                        if i < len(S_GROUPS):
                            s_group(c + 1, S_STARTS[i], S_GROUPS[i])
                        if i >= 1:
                            u_batch(c, S_STARTS[i - 1], S_GROUPS[i - 1])
                    u_final(c)

                # last-chunk endgame. The final chunk's S-matmuls are
                # slot-paced against its own exps (depth-1 PSUM rotation, no
                # following chunk hides the turnaround), and a stalled
                # S-matmul blocks everything behind it in the in-order PE
                # queue. So: finish ALL of U(6) first (it only needs exp(6),
                # which the ACT is producing now), then run the final
                # chunk's S as 16 small groups of 2 with U(7) batches
                # interleaved -- each slot release costs the PE just
                # 0.4+0.5us before the next exp can start.
                c = NC - 2
                for i in range(len(S_GROUPS)):
                    u_batch(c, S_STARTS[i], S_GROUPS[i])
                u_final(c)
                last = NC - 1
                s_group(last, 0, 2, pool=st4, tag="st4")
                s_group(last, 2, 2, pool=st2, tag="st2")
                for i in range(2, 16):
                    pool, tag = (st4, "st4") if i % 2 == 0 else (st2, "st2")
                    s_group(last, i * 2, 2, pool=pool, tag=tag)
                    u_batch(last, (i - 2) * 2, 2)
                u_batch(last, 28, 2)
                u_batch(last, 30, 2)
                u_final(last)

            if reps == 1:
                body()
            else:
                with tc.For_i(0, reps, 1):
                    body()

    nc.compile()
    return nc


def _prep_weights(Wkv, bkv, Wp, bp):
    Wkv = np.asarray(Wkv, np.float32)
    bkv = np.asarray(bkv, np.float32)
    Wp = np.asarray(Wp, np.float32)
    bp = np.asarray(bp, np.float32)
    wk = np.ascontiguousarray(Wkv[:, :D].astype(np.float16))
    bk = np.ascontiguousarray(bkv[:D]).reshape(D, 1)
    wvp = np.ascontiguousarray((Wkv[:, D:] @ Wp).astype(np.float16))
    bpe_row = bkv[D:] @ Wp + bp
    bpe = np.ascontiguousarray(np.tile(bpe_row[None, :], (D, 1)))
    return wk, bk, wvp, bpe


_NC_CACHE = {}


def kernel(x, q_global, Wkv, bkv, Wp, bp):
    xt = np.asarray(x, np.float32).astype(np.float16).transpose(0, 2, 1)
    qt = np.asarray(q_global, np.float32).astype(np.float16).transpose(0, 2, 1)
    wk, bk, wvp, bpe = _prep_weights(Wkv, bkv, Wp, bp)

    if 1 not in _NC_CACHE:
        _NC_CACHE[1] = build(reps=1)
    nc = _NC_CACHE[1]

    in_maps = [
        {
            "xt": np.ascontiguousarray(xt[b]),
            "qt": np.ascontiguousarray(qt[b]),
            "wk": wk,
            "wvp": wvp,
            "bk": bk,
            "bpe": bpe,
        }
        for b in range(B)
    ]
    res = run_bass_kernel_spmd(nc, in_maps, core_ids=list(range(B)))
    return np.stack([res.results[b]["out"] for b in range(B)], axis=0)


# revision 11
# speedup vs baseline: 1.1488x; 1.1488x over previous
"""Trainium2 Bass kernel for nn_AttnGlobal (B=8, N=4096, DIM=128).

reference:
    kv = x @ Wkv + bkv ; k, v = split(kv)
    q = q_global / sqrt(d)
    scores = einsum("bnd,bmd->bnm", k, q)       # softmax over m
    attn = softmax(scores, axis=-1)
    out = einsum("bnm,bmd->bnd", attn, v) @ Wp + bp

Sharding: pure data-parallel over B across the 8 cores (one batch each).

Host-side algebra folds:
    w   = x @ (Wv @ Wp)            (since attn @ (x@Wv) @ Wp = attn @ (x@(Wv@Wp)))
    bpe = bv @ Wp + bp             (since rows of attn sum to 1)

Per-core dataflow:
    xT, qT  : host-pretransposed fp16 inputs        [d, n] / [d, m]
    kT      = Wk.T @ xT + bk                        [d, n]   fp16
    S.T     = qT.T-chunks @ kT                      [m, n] tiles in PSUM (fp32)
    E.T     = exp(S.T / sqrt(d))                    fp16, ACT straight from PSUM
    U_aug   = E @ [w | 1]                           [n, 129] accumulated in PSUM
    out     = U[:, :128] * (1 / U[:, 128]) + bpe    DVE, then DMA out

Schedule: the ACT engine's exp stream (16.8M elems/core @ 1 elem/cyc/lane)
is the hard floor (~133us). The software pipeline keeps ACT gapless:
per chunk c we emit S-groups of chunk c+1 interleaved with U-batches of
chunk c (which consume exp output written one chunk-iteration earlier),
so the PE fills its ACT-slot-wait gaps with U matmuls instead of idling.
DMA triggers live on sync/gpsimd queues (never the scalar engine).
"""

import os
import sys

try:
    import concourse  # noqa: F401  (resolvable via PYTHONPATH on axon images)
except ImportError:
    for _p in ("/opt/trn_rl_repo", os.path.expanduser("~/.axon_site/_ro/trn_rl_repo")):
        if os.path.isdir(_p) and _p not in sys.path:
            sys.path.append(_p)

import numpy as np

import concourse.bacc as bacc
import concourse.mybir as mybir
from concourse.bass_utils import run_bass_kernel_spmd
from concourse.tile import TileContext

B, N, D = 8, 4096, 128
NT = N // 128          # 32 row tiles
NC = N // 512          # 8 column chunks
F32 = mybir.dt.float32
F16 = mybir.dt.float16
EXP_SCALE = 1.0 / float(np.sqrt(D))

# alternating PSUM score-group sizes; sum == NT, st4 uses 4 banks, st2 uses 2
S_GROUPS = [2, 4, 2, 4, 2, 4, 2, 4, 2, 4, 2]
assert sum(S_GROUPS) == NT
S_STARTS = [sum(S_GROUPS[:i]) for i in range(len(S_GROUPS))]


def build(reps: int = 1):
    """Build and compile the per-core Bass program (identical on all cores)."""
    nc = bacc.Bacc("TRN2", target_bir_lowering=False)

    xt = nc.dram_tensor("xt", [D, N], F16, kind="ExternalInput")
    qt = nc.dram_tensor("qt", [D, N], F16, kind="ExternalInput")
    wk = nc.dram_tensor("wk", [D, D], F16, kind="ExternalInput")
    wvp = nc.dram_tensor("wvp", [D, D], F16, kind="ExternalInput")
    bk = nc.dram_tensor("bk", [D, 1], F32, kind="ExternalInput")
    bpe = nc.dram_tensor("bpe", [D, D], F32, kind="ExternalInput")  # row-tiled bias
    out = nc.dram_tensor("out", [N, D], F32, kind="ExternalOutput")

    with TileContext(nc) as tc:
        xTc = [nc.alloc_sbuf_tensor(f"xT{c}", [128, 512], F16) for c in range(NC)]
        qTc = [nc.alloc_sbuf_tensor(f"qT{c}", [128, 512], F16) for c in range(NC)]
        kTc = [nc.alloc_sbuf_tensor(f"kT{c}", [128, 512], F16) for c in range(NC)]
        w_aug = nc.alloc_sbuf_tensor("w_aug", [128, NT, 130], F16)
        ET = [nc.alloc_sbuf_tensor(f"et{i}", [128, NT, 512], F16) for i in range(2)]
        warm_sb = nc.alloc_sbuf_tensor("warm_sb", [128, 128], F16)
        wk_sb = nc.alloc_sbuf_tensor("wk_sb", [128, 128], F16)
        wvp_sb = nc.alloc_sbuf_tensor("wvp_sb", [128, 128], F16)
        bk_sb = nc.alloc_sbuf_tensor("bk_sb", [128, 1], F32)
        bpe_sb = nc.alloc_sbuf_tensor("bpe_sb", [128, 128], F32)

        # weights ride the (idle-at-startup) scalar queue so xt/qt triggers
        # go out first on sync/gpsimd
        nc.scalar.dma_start(wk_sb[:], wk[:])
        nc.scalar.dma_start(bk_sb[:], bk[:])
        nc.scalar.dma_start(wvp_sb[:], wvp[:])
        nc.scalar.dma_start(bpe_sb[:], bpe[:])

        with (
            tc.tile_pool(name="outp", bufs=4) as outp,
            tc.tile_pool(name="small", bufs=4) as small,
            tc.tile_pool(name="ps", bufs=2, space="PSUM") as psh,
            tc.tile_pool(name="st4", bufs=1, space="PSUM") as st4,
            tc.tile_pool(name="st2", bufs=1, space="PSUM") as st2,
        ):
            uacc = {}

            def s_group(c, mt, g, pool=None, tag=None):
                """scores S.T [m-tiles mt..mt+g, n-chunk c] -> exp -> E.T"""
                if pool is None:
                    pool, tag = (st4, "st4") if g == 4 else (st2, "st2")
                stp = pool.tile([128, g * 512], F32, tag=tag)
                for i in range(g):
                    m = mt + i
                    nc.tensor.matmul(
                        stp[:, i * 512:(i + 1) * 512],
                        qTc[m // 4][:, (m % 4) * 128:(m % 4 + 1) * 128],
                        kTc[c][:],
                    )
                nc.scalar.activation(
                    ET[c % 2][:, mt:mt + g, :],
                    stp[:],
                    mybir.ActivationFunctionType.Exp,
                    scale=EXP_SCALE,
                )

            def u_batch(c, mt, g):
                """U += E.T-tiles[mt..mt+g].T @ [w|1] for output chunk c."""
                if c not in uacc:
                    upa = psh.tile([128, 512], F32, tag="ps")
                    upb = psh.tile([128, 512], F32, tag="ps")
                    uacc[c] = (upa, upb)
                ups = uacc[c]
                buf = ET[c % 2]
                for i in range(g):
                    t = mt + i
                    for j in range(4):
                        up = ups[j // 2]
                        off = 129 * (j % 2)
                        nc.tensor.matmul(
                            up[:, off:off + 129],
                            buf[:, t, j * 128:(j + 1) * 128],
                            w_aug[:, t, :129],
                            start=(t == 0 and j % 2 == 0),
                            stop=(t == NT - 1 and j % 2 == 1),
                        )

            def u_final(c):
                """normalize U by its ones-column, add bias, DMA out."""
                ups = uacc.pop(c)
                for j in range(4):
                    up = ups[j // 2]
                    off = 129 * (j % 2)
                    rec = small.tile([128, 1], F32, tag="rec")
                    nc.vector.reciprocal(rec[:], up[:, off + 128:off + 129])
                    ot = outp.tile([128, 128], F32, tag="ot")
                    nc.vector.scalar_tensor_tensor(
                        ot[:],
                        up[:, off:off + 128],
                        rec[:],
                        bpe_sb[:],
                        mybir.AluOpType.mult,
                        mybir.AluOpType.add,
                    )
                    row = c * 512 + j * 128
                    nc.sync.dma_start(out[row:row + 128, :], ot[:])

            def body(_iv=None):
                # HAM warmup: data-independent matmuls into a scratch PSUM
                # slot keep the PE busy while the first input DMAs land, so
                # the 4096-cycle activity window un-throttles the clock gate
                # (K=4/8 -> 8/8) before the real S-matmuls start.
                nc.vector.memset(warm_sb[:], 0.0)
                warm = psh.tile([128, 512], F32, tag="ps")
                for _ in range(6):
                    nc.tensor.matmul(warm[:, :128], warm_sb[:], warm_sb[:])

                # The PE executes its queue in order, and every S-group
                # matmul is slot-paced: S(c,i) can only run once
                # exp(c,i-2) has freed its PSUM slot (depth-1 rotation per
                # pool). That release instant is exactly when U(c,i-2)'s
                # input (the exp output) becomes ready -- so the natural
                # schedule pairs each S-group with the U-batch TWO groups
                # behind it in the SAME chunk. Each ACT window then carries
                # S(w)+U(w) = ~14us of PE work against ~16.5us of exp, for
                # every chunk including the last; the ACT stream never
                # starves and there is no terminal U pile-up.
                nc.vector.memset(w_aug[:, :, 128:129], 1.0)

                def chunk_prologue(c):
                    # inputs for chunk c: xT/qT DMAs, kT, w_aug tiles
                    nc.sync.dma_start(xTc[c][:], xt[:, c * 512:(c + 1) * 512])
                    nc.gpsimd.dma_start(qTc[c][:], qt[:, c * 512:(c + 1) * 512])
                    kt = psh.tile([128, 512], F32, tag="ps")
                    nc.tensor.matmul(kt[:], wk_sb[:], xTc[c][:])
                    nc.vector.tensor_scalar_add(kTc[c][:], kt[:], bk_sb[:])
                    for i in range(4):
                        t = c * 4 + i
                        wp = psh.tile([128, 512], F32, tag="ps")
                        nc.tensor.matmul(
                            wp[:, :128],
                            xTc[c][:, i * 128:(i + 1) * 128],
                            wvp_sb[:],
                        )
                        nc.vector.tensor_copy(w_aug[:, t, :128], wp[:, :128])

                # chunk 0 feeds the pipeline: its S-groups are gated by the
                # progressive qT arrival, with kT/w of later chunks filling
                # PE gaps; from chunk 1 on all inputs are resident. No
                # U-batches here: the accumulator tiles share the "ps" tag
                # with the prologue scratch, so every scratch allocation
                # must precede the first (long-held) accumulator allocation
                # or the in-order PE queue deadlocks on the pool ring.
                sg = 0
                mt_done = 0
                for c in range(NC):
                    chunk_prologue(c)
                    while sg < len(S_GROUPS) and mt_done + S_GROUPS[sg] <= c * 4:
                        s_group(0, mt_done, S_GROUPS[sg])
                        mt_done += S_GROUPS[sg]
                        sg += 1

                for c in range(NC):
                    lo = sg if c == 0 else 0
                    if c == 0:
                        # U-batches whose S-groups were emitted in phase 1
                        for i in range(max(lo - 2, 0)):
                            u_batch(0, S_STARTS[i], S_GROUPS[i])
                    for i in range(lo, len(S_GROUPS)):
                        s_group(c, S_STARTS[i], S_GROUPS[i])
                        if i >= 2:
                            u_batch(c, S_STARTS[i - 2], S_GROUPS[i - 2])
                    for i in range(len(S_GROUPS) - 2, len(S_GROUPS)):
                        u_batch(c, S_STARTS[i], S_GROUPS[i])
                    u_final(c)

            if reps == 1:
                body()
            else:
                with tc.For_i(0, reps, 1):
                    body()

    nc.compile()
    return nc


def _prep_weights(Wkv, bkv, Wp, bp):
    Wkv = np.asarray(Wkv, np.float32)
    bkv = np.asarray(bkv, np.float32)
    Wp = np.asarray(Wp, np.float32)
    bp = np.asarray(bp, np.float32)
    wk = np.ascontiguousarray(Wkv[:, :D].astype(np.float16))
    bk = np.ascontiguousarray(bkv[:D]).reshape(D, 1)
    wvp = np.ascontiguousarray((Wkv[:, D:] @ Wp).astype(np.float16))
    bpe_row = bkv[D:] @ Wp + bp
    bpe = np.ascontiguousarray(np.tile(bpe_row[None, :], (D, 1)))
    return wk, bk, wvp, bpe


_NC_CACHE = {}


def kernel(x, q_global, Wkv, bkv, Wp, bp):
    xt = np.asarray(x, np.float32).astype(np.float16).transpose(0, 2, 1)
    qt = np.asarray(q_global, np.float32).astype(np.float16).transpose(0, 2, 1)
    wk, bk, wvp, bpe = _prep_weights(Wkv, bkv, Wp, bp)

    if 1 not in _NC_CACHE:
        _NC_CACHE[1] = build(reps=1)
    nc = _NC_CACHE[1]

    in_maps = [
        {
            "xt": np.ascontiguousarray(xt[b]),
            "qt": np.ascontiguousarray(qt[b]),
            "wk": wk,
            "wvp": wvp,
            "bk": bk,
            "bpe": bpe,
        }
        for b in range(B)
    ]
    res = run_bass_kernel_spmd(nc, in_maps, core_ids=list(range(B)))
    return np.stack([res.results[b]["out"] for b in range(B)], axis=0)


# revision 12
# speedup vs baseline: 1.1491x; 1.0003x over previous
"""Trainium2 Bass kernel for nn_AttnGlobal (B=8, N=4096, DIM=128).

reference:
    kv = x @ Wkv + bkv ; k, v = split(kv)
    q = q_global / sqrt(d)
    scores = einsum("bnd,bmd->bnm", k, q)       # softmax over m
    attn = softmax(scores, axis=-1)
    out = einsum("bnm,bmd->bnd", attn, v) @ Wp + bp

Sharding: pure data-parallel over B across the 8 cores (one batch each).

Host-side algebra folds:
    w   = x @ (Wv @ Wp)            (since attn @ (x@Wv) @ Wp = attn @ (x@(Wv@Wp)))
    bpe = bv @ Wp + bp             (since rows of attn sum to 1)

Per-core dataflow:
    xT, qT  : host-pretransposed fp16 inputs        [d, n] / [d, m]
    kT      = Wk.T @ xT + bk                        [d, n]   fp16
    S.T     = qT.T-tiles @ kT                       [m, n] tiles in PSUM (fp32)
    E.T     = exp(S.T / sqrt(d))                    fp16, ACT straight from PSUM
    U_aug   = E @ [w | 1]                           [n, 129] accumulated in PSUM
    out     = U[:, :128] * (1 / U[:, 128]) + bpe    DVE, then DMA out

Schedule: the ACT engine's exp stream (16.8M elems/core @ 1 elem/cyc/lane)
is the hard floor (~128us). Everything is arranged to keep ACT gapless:

- S-group matmuls are slot-paced: S(c,i) can only run once exp(c,i-2) has
  freed its PSUM slot (two pools in strict alternation). That release
  instant is exactly when U(c,i-2)'s input is ready, so the emission is
  one flat stream of (S-group[k], U-batch[k-2]) pairs with a GLOBAL index
  crossing chunk boundaries -- each ACT window carries S(w)+U(w) ~14us of
  PE work against ~16us of exp, for every chunk including the last.
- 12 groups per chunk (even count) keep the pool alternation seamless
  across the chunk boundary: the next chunk's first group reuses a slot
  freed two exps earlier, not the immediately preceding one.
- Inputs arrive as two 512KB half-transfers per tensor (8KB contiguous
  lines, separate sync/gpsimd queues); weights ride the scalar queue.
  All PSUM scratch (kT, w tiles) is allocated before the first long-held
  U accumulator in the shared pool ring to keep the in-order PE queue
  deadlock-free.
"""

import os
import sys

try:
    import concourse  # noqa: F401  (resolvable via PYTHONPATH on axon images)
except ImportError:
    for _p in ("/opt/trn_rl_repo", os.path.expanduser("~/.axon_site/_ro/trn_rl_repo")):
        if os.path.isdir(_p) and _p not in sys.path:
            sys.path.append(_p)

import numpy as np

import concourse.bacc as bacc
import concourse.mybir as mybir
from concourse.bass_utils import run_bass_kernel_spmd
from concourse.tile import TileContext

B, N, D = 8, 4096, 128
NT = N // 128          # 32 row tiles
NC = N // 512          # 8 column chunks
F32 = mybir.dt.float32
F16 = mybir.dt.float16
EXP_SCALE = 1.0 / float(np.sqrt(D))

# 12 score-groups per chunk, even index -> st2 pool (2 banks), odd -> st4
# (4 banks); strict alternation incl. across chunk boundaries.
S_GROUPS = [2, 4, 2, 4, 2, 4, 2, 4, 2, 2, 2, 2]
assert sum(S_GROUPS) == NT
S_STARTS = [sum(S_GROUPS[:i]) for i in range(len(S_GROUPS))]
NG = len(S_GROUPS)


def build(reps: int = 1):
    """Build and compile the per-core Bass program (identical on all cores)."""
    nc = bacc.Bacc("TRN2", target_bir_lowering=False)

    xt = nc.dram_tensor("xt", [D, N], F16, kind="ExternalInput")
    qt = nc.dram_tensor("qt", [D, N], F16, kind="ExternalInput")
    wk = nc.dram_tensor("wk", [D, D], F16, kind="ExternalInput")
    wvp = nc.dram_tensor("wvp", [D, D], F16, kind="ExternalInput")
    bk = nc.dram_tensor("bk", [D, 1], F32, kind="ExternalInput")
    bpe = nc.dram_tensor("bpe", [D, D], F32, kind="ExternalInput")  # row-tiled bias
    out = nc.dram_tensor("out", [N, D], F32, kind="ExternalOutput")

    with TileContext(nc) as tc:
        xT = nc.alloc_sbuf_tensor("xT", [128, N], F16)
        qT = nc.alloc_sbuf_tensor("qT", [128, N], F16)
        kTc = [nc.alloc_sbuf_tensor(f"kT{c}", [128, 512], F16) for c in range(NC)]
        w_aug = nc.alloc_sbuf_tensor("w_aug", [128, NT, 130], F16)
        ET = [nc.alloc_sbuf_tensor(f"et{i}", [128, NT, 512], F16) for i in range(2)]
        warm_sb = nc.alloc_sbuf_tensor("warm_sb", [128, 128], F16)
        wk_sb = nc.alloc_sbuf_tensor("wk_sb", [128, 128], F16)
        wvp_sb = nc.alloc_sbuf_tensor("wvp_sb", [128, 128], F16)
        bk_sb = nc.alloc_sbuf_tensor("bk_sb", [128, 1], F32)
        bpe_sb = nc.alloc_sbuf_tensor("bpe_sb", [128, 128], F32)

        H = N // 2
        # halves on separate queues: both tensors fully resident by ~13us
        nc.gpsimd.dma_start(qT[:, :H], qt[:, :H])
        nc.sync.dma_start(xT[:, :H], xt[:, :H])
        nc.gpsimd.dma_start(qT[:, H:], qt[:, H:])
        nc.sync.dma_start(xT[:, H:], xt[:, H:])
        # weights ride the (idle-at-startup) scalar queue
        nc.scalar.dma_start(wk_sb[:], wk[:])
        nc.scalar.dma_start(bk_sb[:], bk[:])
        nc.scalar.dma_start(wvp_sb[:], wvp[:])
        nc.scalar.dma_start(bpe_sb[:], bpe[:])

        with (
            tc.tile_pool(name="outp", bufs=4) as outp,
            tc.tile_pool(name="small", bufs=4) as small,
            tc.tile_pool(name="ps", bufs=2, space="PSUM") as psh,
            tc.tile_pool(name="st4", bufs=1, space="PSUM") as st4,
            tc.tile_pool(name="st2", bufs=1, space="PSUM") as st2,
        ):
            uacc = {}

            def s_group(c, gi):
                """scores S.T [m-tiles of group gi, n-chunk c] -> exp -> E.T"""
                mt, g = S_STARTS[gi], S_GROUPS[gi]
                pool, tag = (st2, "st2") if gi % 2 == 0 else (st4, "st4")
                stp = pool.tile([128, g * 512], F32, tag=tag)
                for i in range(g):
                    m = mt + i
                    nc.tensor.matmul(
                        stp[:, i * 512:(i + 1) * 512],
                        qT[:, m * 128:(m + 1) * 128],
                        kTc[c][:],
                    )
                nc.scalar.activation(
                    ET[c % 2][:, mt:mt + g, :],
                    stp[:],
                    mybir.ActivationFunctionType.Exp,
                    scale=EXP_SCALE,
                )

            def u_batch(c, gi):
                """U += E.T-tiles[group gi].T @ [w | 1] for output chunk c."""
                mt, g = S_STARTS[gi], S_GROUPS[gi]
                if c not in uacc:
                    upa = psh.tile([128, 512], F32, tag="ps")
                    upb = psh.tile([128, 512], F32, tag="ps")
                    uacc[c] = (upa, upb)
                ups = uacc[c]
                buf = ET[c % 2]
                for i in range(g):
                    t = mt + i
                    for j in range(4):
                        up = ups[j // 2]
                        off = 129 * (j % 2)
                        nc.tensor.matmul(
                            up[:, off:off + 129],
                            buf[:, t, j * 128:(j + 1) * 128],
                            w_aug[:, t, :129],
                            start=(t == 0 and j % 2 == 0),
                            stop=(t == NT - 1 and j % 2 == 1),
                        )

            def u_final(c):
                """normalize U by its ones-column, add bias, DMA out."""
                ups = uacc.pop(c)
                for j in range(4):
                    up = ups[j // 2]
                    off = 129 * (j % 2)
                    rec = small.tile([128, 1], F32, tag="rec")
                    nc.vector.reciprocal(rec[:], up[:, off + 128:off + 129])
                    ot = outp.tile([128, 128], F32, tag="ot")
                    nc.vector.scalar_tensor_tensor(
                        ot[:],
                        up[:, off:off + 128],
                        rec[:],
                        bpe_sb[:],
                        mybir.AluOpType.mult,
                        mybir.AluOpType.add,
                    )
                    row = c * 512 + j * 128
                    nc.sync.dma_start(out[row:row + 128, :], ot[:])

            def kT_mm(c):
                kt = psh.tile([128, 512], F32, tag="ps")
                nc.tensor.matmul(kt[:], wk_sb[:], xT[:, c * 512:(c + 1) * 512])
                nc.vector.tensor_scalar_add(kTc[c][:], kt[:], bk_sb[:])

            def w_mm(t):
                wp = psh.tile([128, 512], F32, tag="ps")
                nc.tensor.matmul(
                    wp[:, :128],
                    xT[:, t * 128:(t + 1) * 128],
                    wvp_sb[:],
                )
                nc.vector.tensor_copy(w_aug[:, t, :128], wp[:, :128])

            def body(_iv=None):
                # HAM warmup: data-independent matmuls keep the PE busy
                # while the first input DMAs land, so the clock gate
                # un-throttles (K=4/8 -> 8/8) before the real work starts.
                nc.vector.memset(warm_sb[:], 0.0)
                warm = psh.tile([128, 512], F32, tag="ps")
                for _ in range(6):
                    nc.tensor.matmul(warm[:, :128], warm_sb[:], warm_sb[:])
                nc.vector.memset(w_aug[:, :, 128:129], 1.0)

                # prologue compute: kT(0) unlocks all of S(0); later kT/w
                # fill PE slack inside window 0 (ordered so that everything
                # gated on the second input half sits behind the S-groups
                # that only need the first half). All "ps"-tag scratch is
                # emitted before the first U accumulator allocation.
                kT_mm(0)
                s_group(0, 0)
                s_group(0, 1)
                for c in range(1, 4):
                    kT_mm(c)
                for t in range(0, 16):
                    w_mm(t)
                s_group(0, 2)
                s_group(0, 3)
                s_group(0, 4)       # m-tiles 12..15, last of first qT half
                for c in range(4, NC):
                    kT_mm(c)
                for t in range(16, NT):
                    w_mm(t)

                # flat global-lag-2 pair stream over (chunk, group)
                seq = [(c, gi) for c in range(NC) for gi in range(NG)]
                done = 5            # S(0, 0..4) already emitted
                for k, (c, gi) in enumerate(seq):
                    if k >= done:
                        s_group(c, gi)
                    if k >= 2:
                        uc, ug = seq[k - 2]
                        u_batch(uc, ug)
                        if ug == NG - 1:
                            u_final(uc)
                for k in (len(seq) - 2, len(seq) - 1):
                    uc, ug = seq[k]
                    u_batch(uc, ug)
                    if ug == NG - 1:
                        u_final(uc)

            if reps == 1:
                body()
            else:
                with tc.For_i(0, reps, 1):
                    body()

    nc.compile()
    return nc


def _prep_weights(Wkv, bkv, Wp, bp):
    Wkv = np.asarray(Wkv, np.float32)
    bkv = np.asarray(bkv, np.float32)
    Wp = np.asarray(Wp, np.float32)
    bp = np.asarray(bp, np.float32)
    wk = np.ascontiguousarray(Wkv[:, :D].astype(np.float16))
    bk = np.ascontiguousarray(bkv[:D]).reshape(D, 1)
    wvp = np.ascontiguousarray((Wkv[:, D:] @ Wp).astype(np.float16))
    bpe_row = bkv[D:] @ Wp + bp
    bpe = np.ascontiguousarray(np.tile(bpe_row[None, :], (D, 1)))
    return wk, bk, wvp, bpe


_NC_CACHE = {}


def kernel(x, q_global, Wkv, bkv, Wp, bp):
    xt = np.asarray(x, np.float32).astype(np.float16).transpose(0, 2, 1)
    qt = np.asarray(q_global, np.float32).astype(np.float16).transpose(0, 2, 1)
    wk, bk, wvp, bpe = _prep_weights(Wkv, bkv, Wp, bp)

    if 1 not in _NC_CACHE:
        _NC_CACHE[1] = build(reps=1)
    nc = _NC_CACHE[1]

    in_maps = [
        {
            "xt": np.ascontiguousarray(xt[b]),
            "qt": np.ascontiguousarray(qt[b]),
            "wk": wk,
            "wvp": wvp,
            "bk": bk,
            "bpe": bpe,
        }
        for b in range(B)
    ]
    res = run_bass_kernel_spmd(nc, in_maps, core_ids=list(range(B)))
    return np.stack([res.results[b]["out"] for b in range(B)], axis=0)


# revision 13
# speedup vs baseline: 1.1736x; 1.0213x over previous
"""Trainium2 Bass kernel for nn_AttnGlobal (B=8, N=4096, DIM=128).

reference:
    kv = x @ Wkv + bkv ; k, v = split(kv)
    q = q_global / sqrt(d)
    scores = einsum("bnd,bmd->bnm", k, q)       # softmax over m
    attn = softmax(scores, axis=-1)
    out = einsum("bnm,bmd->bnd", attn, v) @ Wp + bp

Sharding: pure data-parallel over B across the 8 cores (one batch each).

Host-side algebra folds:
    w   = x @ (Wv @ Wp)            (since attn @ (x@Wv) @ Wp = attn @ (x@(Wv@Wp)))
    bpe = bv @ Wp + bp             (since rows of attn sum to 1)

Per-core dataflow:
    xT, qT  : host-pretransposed fp16 inputs        [d, n] / [d, m]
    kT      = Wk.T @ xT + bk                        [d, n]   fp16
    S.T     = qT.T-tiles @ kT                       [m, n] tiles in PSUM (fp32)
    E.T     = exp(S.T / sqrt(d))                    fp16, ACT straight from PSUM
    U_aug   = E @ [w | 1]                           [n, 129] accumulated in PSUM
    out     = U[:, :128] * (1 / U[:, 128]) + bpe    DVE, then DMA out

Schedule: the ACT engine's exp stream (16.8M elems/core @ 1 elem/cyc/lane)
is the hard floor (~128us). Everything else is arranged to keep ACT
gapless:

- S-group matmuls are slot-paced: S(c,i) can only run once exp(c,i-2) has
  freed its PSUM slot (two pools in strict alternation, even group count
  per chunk so the alternation is seamless across chunk boundaries).
  That release instant is exactly when the exp output two groups back
  becomes available, so U-work is emitted as small 2-tile units threaded
  between S-groups at a fixed rate (16 units per 12 slots) by an
  owed-work scheduler.
- Input DMAs are cut into pieces sized to land just before their
  need-time, spread over the sync/gpsimd/scalar queues (~45GB/s each):
  qT alone needs ~63GB/s during window 0 so it is split across two
  queues; w/kT derivations consume xT pieces as they arrive.
- kT and w-tile matmuls borrow the st2/st4 PSUM ring (as inserted
  pseudo-groups) so the accumulator pool ring holds accumulators only --
  the in-order PE queue can never deadlock on pool slots.
"""

import os
import sys

try:
    import concourse  # noqa: F401  (resolvable via PYTHONPATH on axon images)
except ImportError:
    for _p in ("/opt/trn_rl_repo", os.path.expanduser("~/.axon_site/_ro/trn_rl_repo")):
        if os.path.isdir(_p) and _p not in sys.path:
            sys.path.append(_p)

import numpy as np

import concourse.bacc as bacc
import concourse.mybir as mybir
from concourse.bass_utils import run_bass_kernel_spmd
from concourse.tile import TileContext

B, N, D = 8, 4096, 128
NT = N // 128          # 32 row tiles
NC = N // 512          # 8 column chunks
F32 = mybir.dt.float32
F16 = mybir.dt.float16
EXP_SCALE = 1.0 / float(np.sqrt(D))

# 12 score-groups per chunk; even index -> st2 pool (max 2 tiles), odd ->
# st4 (max 4 tiles). Chunk 0 front-loads small groups so the early exps
# only need the first qT DMA pieces.
GROUPS0 = [2, 2, 2, 2, 2, 4, 2, 4, 2, 4, 2, 4]
GROUPSN = [2, 4, 2, 4, 2, 4, 2, 4, 2, 2, 2, 2]
assert sum(GROUPS0) == NT and sum(GROUPSN) == NT
NG = len(GROUPSN)


def _chunk_groups(c):
    sizes = GROUPS0 if c == 0 else GROUPSN
    starts = [sum(sizes[:i]) for i in range(len(sizes))]
    return list(zip(starts, sizes))


def build(reps: int = 1):
    """Build and compile the per-core Bass program (identical on all cores)."""
    nc = bacc.Bacc("TRN2", target_bir_lowering=False)

    xt = nc.dram_tensor("xt", [D, N], F16, kind="ExternalInput")
    qt = nc.dram_tensor("qt", [D, N], F16, kind="ExternalInput")
    wk = nc.dram_tensor("wk", [D, D], F16, kind="ExternalInput")
    wvp = nc.dram_tensor("wvp", [D, D], F16, kind="ExternalInput")
    bk = nc.dram_tensor("bk", [D, 1], F32, kind="ExternalInput")
    bpe = nc.dram_tensor("bpe", [D, D], F32, kind="ExternalInput")  # row-tiled bias
    out = nc.dram_tensor("out", [N, D], F32, kind="ExternalOutput")

    with TileContext(nc) as tc:
        xTc = [nc.alloc_sbuf_tensor(f"xT{c}", [128, 512], F16) for c in range(NC)]
        qTp = [nc.alloc_sbuf_tensor(f"qT{p}", [128, 512], F16) for p in range(NC)]
        kTc = [nc.alloc_sbuf_tensor(f"kT{c}", [128, 512], F16) for c in range(NC)]
        w_aug = nc.alloc_sbuf_tensor("w_aug", [128, NT, 130], F16)
        ET = [nc.alloc_sbuf_tensor(f"et{i}", [128, NT, 512], F16) for i in range(2)]
        warm_sb = nc.alloc_sbuf_tensor("warm_sb", [128, 128], F16)
        wk_sb = nc.alloc_sbuf_tensor("wk_sb", [128, 128], F16)
        wvp_sb = nc.alloc_sbuf_tensor("wvp_sb", [128, 128], F16)
        bk_sb = nc.alloc_sbuf_tensor("bk_sb", [128, 1], F32)
        bpe_sb = nc.alloc_sbuf_tensor("bpe_sb", [128, 128], F32)

        # DMA piece plan (each queue streams sequentially at ~45GB/s;
        # ~2.8us per 128KB piece): qT pieces alternate between gpsimd and
        # scalar so the m-tile arrival rate keeps up with the exp stream;
        # xT rides sync (chunk 0 first for kT(0)), with the last two
        # chunks on gpsimd after its qT pieces are out.
        def xp(c):
            return (xt[:, c * 512:(c + 1) * 512], xTc[c][:])

        def qp(p):
            return (qt[:, p * 512:(p + 1) * 512], qTp[p][:])

        for src, dst in [xp(0), xp(1), xp(2), xp(3), xp(4), xp(5)]:
            nc.sync.dma_start(dst, src)
        for src, dst in [qp(0), qp(2), qp(4), qp(6), xp(6), xp(7)]:
            nc.gpsimd.dma_start(dst, src)
        nc.scalar.dma_start(wk_sb[:], wk[:])
        nc.scalar.dma_start(bk_sb[:], bk[:])
        nc.scalar.dma_start(wvp_sb[:], wvp[:])
        nc.scalar.dma_start(bpe_sb[:], bpe[:])
        for src, dst in [qp(1), qp(3), qp(5), qp(7)]:
            nc.scalar.dma_start(dst, src)

        with (
            tc.tile_pool(name="outp", bufs=4) as outp,
            tc.tile_pool(name="small", bufs=4) as small,
            tc.tile_pool(name="ps", bufs=2, space="PSUM") as psh,
            tc.tile_pool(name="st4", bufs=1, space="PSUM") as st4,
            tc.tile_pool(name="st2", bufs=1, space="PSUM") as st2,
        ):
            uacc = {}

            def s_group(c, mt, g, gi):
                """scores S.T [m-tiles mt..mt+g, n-chunk c] -> exp -> E.T"""
                pool, tag = (st2, "st2") if gi % 2 == 0 else (st4, "st4")
                stp = pool.tile([128, g * 512], F32, tag=tag)
                for i in range(g):
                    m = mt + i
                    nc.tensor.matmul(
                        stp[:, i * 512:(i + 1) * 512],
                        qTp[m // 4][:, (m % 4) * 128:(m % 4 + 1) * 128],
                        kTc[c][:],
                    )
                nc.scalar.activation(
                    ET[c % 2][:, mt:mt + g, :],
                    stp[:],
                    mybir.ActivationFunctionType.Exp,
                    scale=EXP_SCALE,
                )

            def u_unit(c, j):
                """U += E.T-tiles[2j..2j+1].T @ [w | 1] for output chunk c."""
                if c not in uacc:
                    upa = psh.tile([128, 512], F32, tag="ps")
                    upb = psh.tile([128, 512], F32, tag="ps")
                    uacc[c] = (upa, upb)
                ups = uacc[c]
                buf = ET[c % 2]
                for t in (2 * j, 2 * j + 1):
                    for jj in range(4):
                        up = ups[jj // 2]
                        off = 129 * (jj % 2)
                        nc.tensor.matmul(
                            up[:, off:off + 129],
                            buf[:, t, jj * 128:(jj + 1) * 128],
                            w_aug[:, t, :129],
                            start=(t == 0 and jj % 2 == 0),
                            stop=(t == NT - 1 and jj % 2 == 1),
                        )

            def u_final(c):
                """normalize U by its ones-column, add bias, DMA out."""
                ups = uacc.pop(c)
                for j in range(4):
                    up = ups[j // 2]
                    off = 129 * (j % 2)
                    rec = small.tile([128, 1], F32, tag="rec")
                    nc.vector.reciprocal(rec[:], up[:, off + 128:off + 129])
                    ot = outp.tile([128, 128], F32, tag="ot")
                    nc.vector.scalar_tensor_tensor(
                        ot[:],
                        up[:, off:off + 128],
                        rec[:],
                        bpe_sb[:],
                        mybir.AluOpType.mult,
                        mybir.AluOpType.add,
                    )
                    row = c * 512 + j * 128
                    nc.sync.dma_start(out[row:row + 128, :], ot[:])

            def kT_ring(c):
                # kT via the st2 ring (scratch never touches the "ps" ring)
                kt = st2.tile([128, 512], F32, tag="st2")
                nc.tensor.matmul(kt[:], wk_sb[:], xTc[c][:])
                nc.vector.tensor_scalar_add(kTc[c][:], kt[:], bk_sb[:])

            def w_quad(q):
                # w-tiles 4q..4q+3 via the st2 ring; one strided DVE copy
                wp = st2.tile([128, 512], F32, tag="st2")
                for i in range(4):
                    t = 4 * q + i
                    nc.tensor.matmul(
                        wp[:, i * 128:(i + 1) * 128],
                        xTc[q][:, i * 128:(i + 1) * 128],
                        wvp_sb[:],
                    )
                nc.vector.tensor_copy(w_aug[:, 4 * q:4 * q + 4, :128], wp[:])

            def body(_iv=None):
                # HAM warmup: data-independent matmuls keep the PE busy
                # while the first input DMAs land, so the clock gate
                # un-throttles (K=4/8 -> 8/8) before the real work starts.
                nc.vector.memset(warm_sb[:], 0.0)
                warm = st4.tile([128, 512], F32, tag="st4")
                for _ in range(6):
                    nc.tensor.matmul(warm[:, :128], warm_sb[:], warm_sb[:])
                nc.vector.memset(w_aug[:, :, 128:129], 1.0)
                kT_ring(0)

                # flat slot stream: one S-group per slot, U-units owed at
                # 16 per 12 slots, kT/w-quad scratch inserted where their
                # xT pieces have landed.
                seq = [(c, gi) for c in range(NC) for gi in range(NG)]
                tile_group = {}   # (c, mt) -> global slot index of its group
                for k, (c, gi) in enumerate(seq):
                    mt, g = _chunk_groups(c)[gi]
                    for t in range(mt, mt + g):
                        tile_group[(c, t)] = k

                units = [(c, j) for c in range(NC) for j in range(NT // 2)]
                emitted = 0

                def unit_ready(k, wq_done):
                    if emitted >= len(units):
                        return False
                    c, j = units[emitted]
                    if tile_group[(c, 2 * j + 1)] > k - 2:
                        return False
                    if (2 * j + 1) // 4 >= wq_done and c == 0:
                        return False
                    return True

                wq_done = 0
                # w-quad q is inserted at chunk-0 slot 2q+1 (xT piece q
                # has landed by then); quads 6,7 early in chunk 1.
                wq_slot = {2 * q + 1: q for q in range(6)}
                wq_slot[NG + 1] = 6
                wq_slot[NG + 3] = 7

                for k, (c, gi) in enumerate(seq):
                    mt, g = _chunk_groups(c)[gi]
                    s_group(c, mt, g, gi)
                    if k in wq_slot:
                        w_quad(wq_slot[k])
                        wq_done += 1
                    if gi == 6 and c + 1 < NC:
                        kT_ring(c + 1)
                    target = max(0, ((k - 1) * len(units)) // len(seq))
                    while emitted < target and unit_ready(k, wq_done):
                        uc, uj = units[emitted]
                        u_unit(uc, uj)
                        emitted += 1
                        if uj == NT // 2 - 1:
                            u_final(uc)
                while emitted < len(units):
                    uc, uj = units[emitted]
                    u_unit(uc, uj)
                    emitted += 1
                    if uj == NT // 2 - 1:
                        u_final(uc)

            if reps == 1:
                body()
            else:
                with tc.For_i(0, reps, 1):
                    body()

    nc.compile()
    return nc


def _prep_weights(Wkv, bkv, Wp, bp):
    Wkv = np.asarray(Wkv, np.float32)
    bkv = np.asarray(bkv, np.float32)
    Wp = np.asarray(Wp, np.float32)
    bp = np.asarray(bp, np.float32)
    wk = np.ascontiguousarray(Wkv[:, :D].astype(np.float16))
    bk = np.ascontiguousarray(bkv[:D]).reshape(D, 1)
    wvp = np.ascontiguousarray((Wkv[:, D:] @ Wp).astype(np.float16))
    bpe_row = bkv[D:] @ Wp + bp
    bpe = np.ascontiguousarray(np.tile(bpe_row[None, :], (D, 1)))
    return wk, bk, wvp, bpe


_NC_CACHE = {}


def kernel(x, q_global, Wkv, bkv, Wp, bp):
    xt = np.asarray(x, np.float32).astype(np.float16).transpose(0, 2, 1)
    qt = np.asarray(q_global, np.float32).astype(np.float16).transpose(0, 2, 1)
    wk, bk, wvp, bpe = _prep_weights(Wkv, bkv, Wp, bp)

    if 1 not in _NC_CACHE:
        _NC_CACHE[1] = build(reps=1)
    nc = _NC_CACHE[1]

    in_maps = [
        {
            "xt": np.ascontiguousarray(xt[b]),
            "qt": np.ascontiguousarray(qt[b]),
            "wk": wk,
            "wvp": wvp,
            "bk": bk,
            "bpe": bpe,
        }
        for b in range(B)
    ]
    res = run_bass_kernel_spmd(nc, in_maps, core_ids=list(range(B)))
    return np.stack([res.results[b]["out"] for b in range(B)], axis=0)


# revision 17
# speedup vs baseline: 1.2147x; 1.0350x over previous
"""Trainium2 Bass kernel for nn_AttnGlobal (B=8, N=4096, DIM=128).

reference:
    kv = x @ Wkv + bkv ; k, v = split(kv)
    q = q_global / sqrt(d)
    scores = einsum("bnd,bmd->bnm", k, q)       # softmax over m
    attn = softmax(scores, axis=-1)
    out = einsum("bnm,bmd->bnd", attn, v) @ Wp + bp

Sharding: pure data-parallel over B across the 8 cores (one batch each).

Host-side algebra folds:
    w   = x @ (Wv @ Wp)            (since attn @ (x@Wv) @ Wp = attn @ (x@(Wv@Wp)))
    bpe = bv @ Wp + bp             (since rows of attn sum to 1)

Per-core dataflow:
    xT, qT  : host-pretransposed fp16 inputs        [d, n] / [d, m]
    kT      = Wk.T @ xT + bk                        [d, n]   fp16
    S.T     = qT.T-tiles @ kT                       [m, n] tiles in PSUM (fp32)
    E.T     = exp(S.T / sqrt(d))                    fp16, ACT straight from PSUM
    U_aug   = E @ [w | 1]                           [n, 129] accumulated in PSUM
    out     = U[:, :128] * (1 / U[:, 128]) + bpe    DVE, then DMA out

Schedule: the ACT engine's exp stream (16.8M elems/core @ 1 elem/cyc/lane)
is the hard floor (~128us). Everything else is arranged to keep ACT
gapless:

- S-group matmuls are slot-paced: S(c,i) can only run once exp(c,i-2) has
  freed its PSUM slot (two pools in strict alternation, even group count
  per chunk so the alternation is seamless across chunk boundaries).
  That release instant is exactly when the exp output two groups back
  becomes available, so U-work is emitted as small 2-tile units threaded
  between S-groups at a fixed rate (16 units per 12 slots) by an
  owed-work scheduler.
- Input DMAs are cut into pieces sized to land just before their
  need-time, spread over the sync/gpsimd/scalar queues (~45GB/s each):
  qT alone needs ~63GB/s during window 0 so it is split across two
  queues; w/kT derivations consume xT pieces as they arrive.
- kT and w-tile matmuls borrow the st2/st4 PSUM ring (as inserted
  pseudo-groups) so the accumulator pool ring holds accumulators only --
  the in-order PE queue can never deadlock on pool slots.
"""

import os
import sys

try:
    import concourse  # noqa: F401  (resolvable via PYTHONPATH on axon images)
except ImportError:
    for _p in ("/opt/trn_rl_repo", os.path.expanduser("~/.axon_site/_ro/trn_rl_repo")):
        if os.path.isdir(_p) and _p not in sys.path:
            sys.path.append(_p)

import numpy as np

import concourse.bacc as bacc
import concourse.mybir as mybir
from concourse.bass_utils import run_bass_kernel_spmd
from concourse.tile import TileContext

B, N, D = 8, 4096, 128
NT = N // 128          # 32 row tiles
NC = N // 512          # 8 column chunks
F32 = mybir.dt.float32
F16 = mybir.dt.float16
EXP_SCALE = 1.0 / float(np.sqrt(D))

# 12 score-groups per chunk; even index -> st2 pool (max 2 tiles), odd ->
# st4 (max 4 tiles). Small groups lead each chunk so chunk 0's early exps
# only need the first qT DMA pieces; the even count keeps the st2/st4
# alternation seamless across chunk boundaries.
GROUPS = [2, 2, 2, 2, 2, 4, 2, 4, 2, 4, 2, 4]
assert sum(GROUPS) == NT
NG = len(GROUPS)
GSTART = [sum(GROUPS[:i]) for i in range(NG)]


def _chunk_groups(c):
    return list(zip(GSTART, GROUPS))


def build(reps: int = 1):
    """Build and compile the per-core Bass program (identical on all cores)."""
    nc = bacc.Bacc("TRN2", target_bir_lowering=False)

    xt = nc.dram_tensor("xt", [D, N], F16, kind="ExternalInput")
    qt = nc.dram_tensor("qt", [D, N], F16, kind="ExternalInput")
    wk = nc.dram_tensor("wk", [D, D], F16, kind="ExternalInput")
    wvp = nc.dram_tensor("wvp", [D, D], F16, kind="ExternalInput")
    bk = nc.dram_tensor("bk", [D, 1], F32, kind="ExternalInput")
    bpe = nc.dram_tensor("bpe", [D, D], F32, kind="ExternalInput")  # row-tiled bias
    out = nc.dram_tensor("out", [N, D], F32, kind="ExternalOutput")

    with TileContext(nc) as tc:
        xTc = [nc.alloc_sbuf_tensor(f"xT{c}", [128, 512], F16) for c in range(NC)]
        qTp = [nc.alloc_sbuf_tensor(f"qT{p}", [128, 512], F16) for p in range(NC)]
        kTc = [nc.alloc_sbuf_tensor(f"kT{c}", [128, 512], F16) for c in range(NC)]
        w_aug = nc.alloc_sbuf_tensor("w_aug", [128, NT, 130], F16)
        ET = [nc.alloc_sbuf_tensor(f"et{i}", [128, NT, 512], F16) for i in range(3)]
        warm_sb = nc.alloc_sbuf_tensor("warm_sb", [128, 128], F16)
        wk_sb = nc.alloc_sbuf_tensor("wk_sb", [128, 128], F16)
        wvp_sb = nc.alloc_sbuf_tensor("wvp_sb", [128, 128], F16)
        bk_sb = nc.alloc_sbuf_tensor("bk_sb", [128, 1], F32)
        bpe_sb = nc.alloc_sbuf_tensor("bpe_sb", [128, 128], F32)

        # DMA piece plan (each queue streams sequentially at ~45GB/s;
        # ~2.8us per 128KB piece): qT pieces alternate between gpsimd and
        # scalar so the m-tile arrival rate keeps up with the exp stream;
        # xT rides sync (chunk 0 first for kT(0)), with the last two
        # chunks on gpsimd after its qT pieces are out.
        def xp(c):
            return (xt[:, c * 512:(c + 1) * 512], xTc[c][:])

        def qp(p):
            return (qt[:, p * 512:(p + 1) * 512], qTp[p][:])

        for src, dst in [xp(0), xp(1), xp(2), xp(3), xp(4), xp(5)]:
            nc.sync.dma_start(dst, src)
        for src, dst in [qp(0), qp(2), qp(4), qp(6), xp(6), xp(7)]:
            nc.gpsimd.dma_start(dst, src)
        nc.scalar.dma_start(wk_sb[:], wk[:])
        nc.scalar.dma_start(bk_sb[:], bk[:])
        nc.scalar.dma_start(wvp_sb[:], wvp[:])
        nc.scalar.dma_start(bpe_sb[:], bpe[:])
        for src, dst in [qp(1), qp(3), qp(5), qp(7)]:
            nc.scalar.dma_start(dst, src)

        with (
            tc.tile_pool(name="outp", bufs=4) as outp,
            tc.tile_pool(name="small", bufs=4) as small,
            tc.tile_pool(name="ps", bufs=2, space="PSUM") as psh,
            tc.tile_pool(name="st4", bufs=1, space="PSUM") as st4,
            tc.tile_pool(name="st2", bufs=1, space="PSUM") as st2,
        ):
            uacc = {}

            def s_group(c, mt, g, gi):
                """scores S.T [m-tiles mt..mt+g, n-chunk c] -> exp -> E.T"""
                pool, tag = (st2, "st2") if gi % 2 == 0 else (st4, "st4")
                stp = pool.tile([128, g * 512], F32, tag=tag)
                for i in range(g):
                    m = mt + i
                    nc.tensor.matmul(
                        stp[:, i * 512:(i + 1) * 512],
                        qTp[m // 4][:, (m % 4) * 128:(m % 4 + 1) * 128],
                        kTc[c][:],
                    )
                nc.scalar.activation(
                    ET[c % 3][:, mt:mt + g, :],
                    stp[:],
                    mybir.ActivationFunctionType.Exp,
                    scale=EXP_SCALE,
                )

            def u_unit(c, j):
                """U += E.T-tiles[2j..2j+1].T @ [w | 1] for output chunk c."""
                if c not in uacc:
                    upa = psh.tile([128, 512], F32, tag="ps")
                    upb = psh.tile([128, 512], F32, tag="ps")
                    uacc[c] = (upa, upb)
                ups = uacc[c]
                buf = ET[c % 3]
                for t in (2 * j, 2 * j + 1):
                    for jj in range(4):
                        up = ups[jj // 2]
                        off = 129 * (jj % 2)
                        nc.tensor.matmul(
                            up[:, off:off + 129],
                            buf[:, t, jj * 128:(jj + 1) * 128],
                            w_aug[:, t, :129],
                            start=(t == 0 and jj % 2 == 0),
                            stop=(t == NT - 1 and jj % 2 == 1),
                        )

            def u_final(c):
                """normalize U by its ones-column, add bias, DMA out."""
                ups = uacc.pop(c)
                for j in range(4):
                    up = ups[j // 2]
                    off = 129 * (j % 2)
                    rec = small.tile([128, 1], F32, tag="rec")
                    nc.vector.reciprocal(rec[:], up[:, off + 128:off + 129])
                    ot = outp.tile([128, 128], F32, tag="ot")
                    nc.vector.scalar_tensor_tensor(
                        ot[:],
                        up[:, off:off + 128],
                        rec[:],
                        bpe_sb[:],
                        mybir.AluOpType.mult,
                        mybir.AluOpType.add,
                    )
                    row = c * 512 + j * 128
                    nc.sync.dma_start(out[row:row + 128, :], ot[:])

            def kT_ps(c):
                kt = psh.tile([128, 512], F32, tag="ps")
                nc.tensor.matmul(kt[:], wk_sb[:], xTc[c][:])
                nc.vector.tensor_scalar_add(kTc[c][:], kt[:], bk_sb[:])

            def w_quad(q):
                # w-tiles 4q..4q+3; one strided DVE copy into w_aug
                wp = psh.tile([128, 512], F32, tag="ps")
                for i in range(4):
                    t = 4 * q + i
                    nc.tensor.matmul(
                        wp[:, i * 128:(i + 1) * 128],
                        xTc[q][:, i * 128:(i + 1) * 128],
                        wvp_sb[:],
                    )
                nc.vector.tensor_copy(w_aug[:, 4 * q:4 * q + 4, :128], wp[:])

            def body(_iv=None):
                # HAM warmup: data-independent matmuls keep the PE busy
                # while the first input DMAs land, so the clock gate
                # un-throttles (K=4/8 -> 8/8) before the real work starts.
                nc.vector.memset(warm_sb[:], 0.0)
                warm = psh.tile([128, 512], F32, tag="ps")
                for _ in range(6):
                    nc.tensor.matmul(warm[:, :128], warm_sb[:], warm_sb[:])
                nc.vector.memset(w_aug[:, :, 128:129], 1.0)
                kT_ps(0)

                # All kT/w-quad scratch is threaded through chunk 0's slot
                # stream: window 0's exp pace is DMA-arrival-bound anyway,
                # so the scratch matmuls execute inside its slack, each
                # placed at a slot by which its xT piece has landed. U-units
                # are deferred (ACT-paced target starting at slot 6), which
                # also guarantees every "ps"-ring scratch allocation
                # precedes the first long-held U accumulator: the in-order
                # PE queue can never deadlock on the pool ring.
                scratch = {
                    1: [lambda: w_quad(0)],
                    2: [lambda: kT_ps(1)],
                    3: [lambda: w_quad(1)],
                    4: [lambda: kT_ps(2)],
                    5: [lambda: w_quad(2)],
                    6: [lambda: kT_ps(3)],
                    7: [lambda: w_quad(3)],
                    8: [lambda: kT_ps(4), lambda: w_quad(4)],
                    9: [lambda: kT_ps(6), lambda: w_quad(6)],
                    10: [lambda: kT_ps(5), lambda: w_quad(5)],
                    11: [lambda: kT_ps(7), lambda: w_quad(7)],
                }

                seq = [(c, gi) for c in range(NC) for gi in range(NG)]
                tile_group = {}   # (c, mt) -> global slot index of its group
                for k, (c, gi) in enumerate(seq):
                    mt, g = _chunk_groups(c)[gi]
                    for t in range(mt, mt + g):
                        tile_group[(c, t)] = k

                units = [(c, j) for c in range(NC) for j in range(NT // 2)]
                emitted = 0

                def unit_ready(k):
                    if emitted >= len(units):
                        return False
                    c, j = units[emitted]
                    return tile_group[(c, 2 * j + 1)] <= k - 2

                def emit_unit():
                    nonlocal emitted
                    uc, uj = units[emitted]
                    u_unit(uc, uj)
                    emitted += 1
                    if uj == NT // 2 - 1:
                        u_final(uc)

                for k, (c, gi) in enumerate(seq):
                    mt, g = _chunk_groups(c)[gi]
                    s_group(c, mt, g, gi)
                    for fn in scratch.get(k, ()):
                        fn()
                    target = min(len(units), max(0, ((k - 11) * 128) // 85))
                    while emitted < target and unit_ready(k):
                        emit_unit()
                while emitted < len(units):
                    emit_unit()

            if reps == 1:
                body()
            else:
                with tc.For_i(0, reps, 1):
                    body()

    nc.compile()
    return nc


def _prep_weights(Wkv, bkv, Wp, bp):
    Wkv = np.asarray(Wkv, np.float32)
    bkv = np.asarray(bkv, np.float32)
    Wp = np.asarray(Wp, np.float32)
    bp = np.asarray(bp, np.float32)
    wk = np.ascontiguousarray(Wkv[:, :D].astype(np.float16))
    bk = np.ascontiguousarray(bkv[:D]).reshape(D, 1)
    wvp = np.ascontiguousarray((Wkv[:, D:] @ Wp).astype(np.float16))
    bpe_row = bkv[D:] @ Wp + bp
    bpe = np.ascontiguousarray(np.tile(bpe_row[None, :], (D, 1)))
    return wk, bk, wvp, bpe


_NC_CACHE = {}


def kernel(x, q_global, Wkv, bkv, Wp, bp):
    xt = np.asarray(x, np.float32).astype(np.float16).transpose(0, 2, 1)
    qt = np.asarray(q_global, np.float32).astype(np.float16).transpose(0, 2, 1)
    wk, bk, wvp, bpe = _prep_weights(Wkv, bkv, Wp, bp)

    if 1 not in _NC_CACHE:
        _NC_CACHE[1] = build(reps=1)
    nc = _NC_CACHE[1]

    in_maps = [
        {
            "xt": np.ascontiguousarray(xt[b]),
            "qt": np.ascontiguousarray(qt[b]),
            "wk": wk,
            "wvp": wvp,
            "bk": bk,
            "bpe": bpe,
        }
        for b in range(B)
    ]
    res = run_bass_kernel_spmd(nc, in_maps, core_ids=list(range(B)))
    return np.stack([res.results[b]["out"] for b in range(B)], axis=0)
